# revision 5
# baseline (speedup 1.0000x reference)
"""MoE ConditionalFeedForward (int8 SwiGLU experts) on 8 trn2 NeuronCores.

Expert-parallel: host routes token(+slot) pairs to their expert, pads each
expert's batch to a common capacity C, pre-tiles the int8 weights into the
exact contiguous chunks the kernel DMAs, and ships one expert per core.

All three weight matrices ship as int8 (44MB/core of HBM reads).  fp16
copies for the PE are produced three ways, balanced so that the DMA write
fabric (~400GB/s), the HBM read side (~360GB/s) and the ACT/DVE/POOL
engines all stay below the PE's matmul floor (~2688 MMs x (C/2.4+2.5)ns):

  - w1: SWDGE DMA-cast int8->fp16 in flight (measured ~full write rate)
  - w3: split per group: engine casts (ACT/DVE/POOL greedy) / DMA-cast
  - w2: staged just-in-time during phase B, engine/DMA-cast split

Phase A computes, per pair of i-tiles (both accumulated in ONE 2KB psum
bank as [P,2,C]):
    t1 = p1 * s1     (broadcast mul, DVE/POOL)
    a  = Silu(t1)    (ACT, batched over both tiles)
    t3 = p3 * s3     (broadcast mul)
    h  = t3 * a      (DVE/POOL)
Phase B: y^T[m,c] = sum_i w2[m,i] h^T[i,c] with 4 psum banks (pbm=4
m-tiles concurrent), scale by s2 on DVE, DMA out per m-tile.
"""

import os

import numpy as np

os.environ.setdefault("JAX_COMPILATION_CACHE_DIR", "/tmp/jax_cache")

# Problem constants (hardcoded per the task contract).
E = 8
D = 2048
I = 7168
P = 128

KD = D // P              # 16 contraction tiles for GEMM1/3
KI = I // P              # 56 i tiles
MT = D // P              # 16 output m tiles
PBM = 4                  # phase B m-tiles in flight (PSUM banks)
PBW = PBM * P            # 512: phase B weight chunk width (m cols)
MH = MT // PBM           # 4 phase-B m-groups
PBI = 8                  # phase B i-tiles per staged chunk
NB = KI // PBI           # 7 chunks per m-group

# phase A i-group sizes (first groups small for fast start)
GWS = [128, 384] + [512] * 13
assert sum(GWS) == I
# per-group w3 handling: True -> engine-cast (stage int8), False -> DMA-cast
W3_ENG = [False, False, True, True, False, True, True, False, True, True,
          False, True, True, False, True]
assert len(W3_ENG) == len(GWS)
# per-(mh,nb) w2 handling: True -> engine-cast
W2_ENG = [[(nb + mh) % 2 == 0 for nb in range(NB)] for mh in range(MH)]

_CACHE = {}
_LAST_RESULTS = None  # for test harness introspection


def _build_nc(C):
    import contextlib

    import concourse.bacc as bacc
    import concourse.tile as tile
    from concourse import mybir

    f16 = mybir.dt.float16
    f32 = mybir.dt.float32
    i8 = mybir.dt.int8

    assert C <= 512
    jn_max = 2 if C <= 256 else 1

    nc = bacc.Bacc("TRN2", target_bir_lowering=False, debug=False, num_devices=E)

    xt = nc.dram_tensor("xt", [P, KD * C], f16, kind="ExternalInput").ap()
    # group-major partition-major int8 weights; per group g the block is
    # [P, KD*gw] with value[p, k*gw+f] = w[gstart+f, k*P+p]
    w1t = nc.dram_tensor("w1t", [P, KD * I], i8, kind="ExternalInput").ap()
    w3t = nc.dram_tensor("w3t", [P, KD * I], i8, kind="ExternalInput").ap()
    # phase B: [MH, P, KI*PBW], value[mh, p, i*PBW+f] = w2[mh*PBW+f, i*P+p]
    w2t = nc.dram_tensor("w2t", [MH, P, KI * PBW], i8, kind="ExternalInput").ap()
    s1 = nc.dram_tensor("s1", [P, KI], f32, kind="ExternalInput").ap()
    s3 = nc.dram_tensor("s3", [P, KI], f32, kind="ExternalInput").ap()
    s2 = nc.dram_tensor("s2", [P, MT], f32, kind="ExternalInput").ap()
    yt = nc.dram_tensor("yt", [D, C], f32, kind="ExternalOutput").ap()

    with tile.TileContext(nc) as tc:
        # greedy engine picker balancing accumulated busy-ns per engine.
        # measured int8->fp16 cast rates ~44-51 el/ns; fp16 muls ~90 el/ns
        # on DVE; POOL also pays ~1us per SWDGE dma trigger (added below).
        acc = {"act": 0.0, "dve": 0.0, "pool": 0.0}

        def pick(cost_ns, engines):
            best = min(engines, key=lambda e: acc[e] + cost_ns[e])
            acc[best] += cost_ns[best]
            return best

        def cast(out, in_, engines=("act", "dve", "pool")):
            elems = 128 * out.free_size()
            cost = {"act": elems / 44 + 250, "dve": elems / 51 + 250,
                    "pool": elems / 50 + 350}
            eng = pick(cost, engines)
            if eng == "act":
                nc.scalar.copy(out, in_)
            elif eng == "dve":
                nc.vector.tensor_copy(out, in_)
            else:
                nc.gpsimd.tensor_copy(out, in_)

        def mul(out, a, b, engines=("dve", "pool")):
            elems = 128 * out.free_size()
            cost = {"dve": elems / 90 + 250, "pool": elems / 50 + 350}
            eng = pick(cost, engines)
            if eng == "dve":
                nc.vector.tensor_mul(out, a, b)
            else:
                nc.gpsimd.tensor_mul(out, a, b)

        def psmul(out, pin, svec, i, jn):
            # out[:, j, :] = pin[:, j, :] * svec[:, i+j]; pin is PSUM so only
            # DVE (batched tensor_mul w/ broadcast) or ACT (per-j scalar.mul)
            elems = 128 * jn * out.shape[2]
            cost = {"dve": elems / 90 + 250,
                    "act": jn * (elems / jn / 44 + 400)}
            eng = pick(cost, ("dve", "act"))
            if eng == "dve":
                nc.vector.tensor_mul(
                    out, pin,
                    svec[:, i:i + jn]
                    .rearrange("p (k o) -> p k o", o=1)
                    .broadcast_to([P, jn, out.shape[2]]))
            else:
                for j in range(jn):
                    nc.scalar.mul(out[:, j, :], pin[:, j, :],
                                  svec[:, i + j:i + j + 1])

        def bview(s, i, jn):
            # [P, jn] slice of a scale vector -> [P, jn, C] 0-stride view
            return (s[:, i:i + jn]
                    .rearrange("p (k o) -> p k o", o=1)
                    .broadcast_to([P, jn, C]))

        with contextlib.ExitStack() as ctx:
            constp = ctx.enter_context(tc.tile_pool(name="const", bufs=1))
            w1p = ctx.enter_context(tc.tile_pool(name="w1p", bufs=3))
            w3sp = ctx.enter_context(tc.tile_pool(name="w3s", bufs=2))
            w3fp = ctx.enter_context(tc.tile_pool(name="w3f", bufs=2))
            hp = ctx.enter_context(tc.tile_pool(name="h", bufs=1))
            ep = ctx.enter_context(tc.tile_pool(name="eltw", bufs=3))
            w2sp = ctx.enter_context(tc.tile_pool(name="w2s", bufs=4))
            w2fp = ctx.enter_context(tc.tile_pool(name="w2f", bufs=4))
            outp = ctx.enter_context(tc.tile_pool(name="outp", bufs=4))

            # constants: x^T fp16 (sync queue, first) and scale vectors
            # (scalar queue so they don't delay x / first weight group).
            xts = constp.tile([P, KD, C], f16)
            nc.sync.dma_start(xts, xt.rearrange("p (k c) -> p k c", c=C))
            s1s = constp.tile([P, KI], f32)
            nc.scalar.dma_start(s1s, s1)
            s3s = constp.tile([P, KI], f32)
            nc.scalar.dma_start(s3s, s3)
            s2s = constp.tile([P, MT], f32)
            nc.scalar.dma_start(s2s, s2)

            h_index = {}   # i-tile index -> (tile, j)
            GWMAX = max(GWS)
            JSZ = jn_max

            # ---------------- Phase A: h = silu(x@w1^T * s1) * (x@w3^T * s3)
            with tc.tile_pool(name="psA", bufs=3, space="PSUM") as psA:
                goff = 0
                for g, gw in enumerate(GWS):
                    il0 = goff // P
                    nil = gw // P
                    # w1 group: DMA-cast int8 -> fp16 (SWDGE)
                    w1f = w1p.tile([P, KD, GWMAX], f16, tag="w1f")
                    nc.gpsimd.dma_start(
                        w1f[:, :, :gw],
                        w1t[:, goff * KD:(goff + gw) * KD]
                        .rearrange("p (k f) -> p k f", f=gw))
                    acc["pool"] += 1000  # SWDGE trigger cost on Q7
                    # w3 group: engine-cast or DMA-cast
                    w3f = w3fp.tile([P, KD, GWMAX], f16, tag="w3f")
                    if W3_ENG[g]:
                        w3s8 = w3sp.tile([P, KD, GWMAX], i8, tag="w3s8")
                        nc.sync.dma_start(
                            w3s8[:, :, :gw],
                            w3t[:, goff * KD:(goff + gw) * KD]
                            .rearrange("p (k f) -> p k f", f=gw))
                        for k in range(KD):
                            cast(w3f[:, k, :gw], w3s8[:, k, :gw])
                    else:
                        nc.gpsimd.dma_start(
                            w3f[:, :, :gw],
                            w3t[:, goff * KD:(goff + gw) * KD]
                            .rearrange("p (k f) -> p k f", f=gw))
                        acc["pool"] += 1000

                    il = 0
                    while il < nil:
                        jn = min(jn_max, nil - il)
                        i = il0 + il
                        p1 = psA.tile([P, JSZ, C], f32, tag="p1")
                        p3 = psA.tile([P, JSZ, C], f32, tag="p3")
                        for j in range(jn):
                            lo = (il + j) * P
                            for k in range(KD):
                                nc.tensor.matmul(
                                    p1[:, j, :], w1f[:, k, lo:lo + P],
                                    xts[:, k, :],
                                    start=(k == 0), stop=(k == KD - 1))
                        for j in range(jn):
                            lo = (il + j) * P
                            for k in range(KD):
                                nc.tensor.matmul(
                                    p3[:, j, :], w3f[:, k, lo:lo + P],
                                    xts[:, k, :],
                                    start=(k == 0), stop=(k == KD - 1))
                        t1 = ep.tile([P, JSZ, C], f16, tag="t1")
                        psmul(t1[:, :jn, :], p1[:, :jn, :], s1s, i, jn)
                        a = ep.tile([P, JSZ, C], f16, tag="a")
                        nc.scalar.activation(
                            a[:, :jn, :], t1[:, :jn, :],
                            mybir.ActivationFunctionType.Silu)
                        acc["act"] += 128 * jn * C / 55 + 400
                        t3 = ep.tile([P, JSZ, C], f16, tag="t3")
                        psmul(t3[:, :jn, :], p3[:, :jn, :], s3s, i, jn)
                        htile = hp.tile([P, JSZ, C], f16, tag=f"h{i}")
                        mul(htile[:, :jn, :], t3[:, :jn, :], a[:, :jn, :])
                        for j in range(jn):
                            h_index[i + j] = (htile, j)
                        il += jn
                    goff += gw

            # ---------------- Phase B: y^T = (w2 @ h) * s2
            with tc.tile_pool(name="psB", bufs=1, space="PSUM") as psB:
                for mh in range(MH):
                    pbs = [psB.tile([P, C], f32, tag=f"pb{ml}",
                                    name=f"pb{mh}_{ml}")
                           for ml in range(PBM)]
                    for nb in range(NB):
                        sl = slice(nb * PBI * PBW, (nb + 1) * PBI * PBW)
                        if W2_ENG[mh][nb]:
                            w2s8 = w2sp.tile([P, PBI, PBW], i8, tag="w2s8")
                            nc.sync.dma_start(
                                w2s8,
                                w2t[mh][:, sl]
                                .rearrange("p (i f) -> p i f", f=PBW))
                            w2f = w2fp.tile([P, PBI, PBW], f16, tag="w2f")
                            for i_l in range(PBI):
                                cast(w2f[:, i_l, :], w2s8[:, i_l, :])
                        else:
                            w2f = w2fp.tile([P, PBI, PBW], f16, tag="w2f")
                            nc.gpsimd.dma_start(
                                w2f,
                                w2t[mh][:, sl]
                                .rearrange("p (i f) -> p i f", f=PBW))
                            acc["pool"] += 1000
                        for i_l in range(PBI):
                            i = nb * PBI + i_l
                            ht, j = h_index[i]
                            for ml in range(PBM):
                                nc.tensor.matmul(
                                    pbs[ml],
                                    w2f[:, i_l, ml * P:(ml + 1) * P],
                                    ht[:, j, :],
                                    start=(i == 0), stop=(i == KI - 1))
                    for ml in range(PBM):
                        m = mh * PBM + ml
                        o = outp.tile([P, C], f32, tag="o")
                        nc.vector.tensor_scalar_mul(o, pbs[ml], s2s[:, m:m + 1])
                        nc.sync.dma_start(yt[m * P:(m + 1) * P, :], o)

    nc.compile()
    return nc


def _tile_w13(w):
    """[I, D] int8 -> [P, KD*I] group-major partition-major chunks."""
    out = np.empty((P, KD * I), dtype=np.int8)
    goff = 0
    for gw in GWS:
        blk = w[goff:goff + gw, :]                      # [gw, D]
        # [p, k*gw + f] = blk[f, k*P + p]
        t = blk.reshape(gw, KD, P).transpose(2, 1, 0)   # [P, KD, gw]
        out[:, goff * KD:(goff + gw) * KD] = t.reshape(P, KD * gw)
        goff += gw
    return np.ascontiguousarray(out)


def _tile_w2(w):
    """[D, I] int8 -> [MH, P, KI*PBW] partition-major."""
    return np.ascontiguousarray(
        w.reshape(MH, PBW, KI, P).transpose(0, 3, 2, 1)
    ).reshape(MH, P, KI * PBW)


def _route(expert_indices):
    idx = np.asarray(expert_indices).astype(np.int64)
    toks, slots = [], []
    for e in range(E):
        t, a = np.nonzero(idx == e)
        toks.append(t)
        slots.append(a)
    return toks, slots


def _prepare(inputs):
    x = np.asarray(inputs["x"], dtype=np.float32)          # [T, D]
    expert_indices = np.asarray(inputs["expert_indices"])  # [T, A]
    w1 = np.asarray(inputs["w1"])                          # [E, I, D] int8
    w2 = np.asarray(inputs["w2"])                          # [E, D, I] int8
    w3 = np.asarray(inputs["w3"])                          # [E, I, D] int8
    scales1 = np.asarray(inputs["scales1"], dtype=np.float32)
    scales2 = np.asarray(inputs["scales2"], dtype=np.float32)
    scales3 = np.asarray(inputs["scales3"], dtype=np.float32)

    T, A = expert_indices.shape
    toks, slots = _route(expert_indices)
    counts = [len(t) for t in toks]
    C = max(max(counts), 8)
    C = (C + 1) // 2 * 2

    if C not in _CACHE:
        _CACHE[C] = _build_nc(C)
    nc = _CACHE[C]

    in_maps = []
    for e in range(E):
        n_e = counts[e]
        # xt layout [P, KD*C]: [p, k*C + c] = x[tok_c, k*P + p]
        xtc = np.zeros((P, KD, C), dtype=np.float16)
        if n_e:
            xe = x[toks[e]].astype(np.float16)             # [n_e, D]
            xtc[:, :, :n_e] = xe.T.reshape(KD, P, n_e).transpose(1, 0, 2)
        in_maps.append(
            dict(
                xt=np.ascontiguousarray(xtc.reshape(P, KD * C)),
                w1t=_tile_w13(w1[e]),
                w3t=_tile_w13(w3[e]),
                w2t=_tile_w2(w2[e]),
                s1=np.ascontiguousarray(scales1[e].reshape(KI, P).T),
                s3=np.ascontiguousarray(scales3[e].reshape(KI, P).T),
                s2=np.ascontiguousarray(scales2[e].reshape(MT, P).T),
            )
        )
    return nc, in_maps, (T, A, toks, slots, counts)


def kernel(**inputs):
    global _LAST_RESULTS
    from concourse.bass_utils import run_bass_kernel_spmd

    nc, in_maps, (T, A, toks, slots, counts) = _prepare(inputs)
    res = run_bass_kernel_spmd(nc, in_maps, core_ids=list(range(E)))
    _LAST_RESULTS = res

    out = np.zeros((T, A, D), dtype=np.float32)
    for e in range(E):
        n_e = counts[e]
        if n_e:
            ye = res.results[e]["yt"][:, :n_e].T  # [n_e, D]
            out[toks[e], slots[e], :] = ye
    return out


# revision 6
# speedup vs baseline: 1.0522x; 1.0522x over previous
"""MoE ConditionalFeedForward (int8 SwiGLU experts) on 8 trn2 NeuronCores.

Expert-parallel: host routes token(+slot) pairs to their expert, pads each
expert's batch to a common capacity C, pre-tiles the weights into the exact
contiguous chunks the kernel DMAs, and ships one expert per core.

The PE matmul floor is 2688 MMs x (C/2.4+2.5)ns ~= 177us at C=152; the
binding resource around it is weight ingest: 44.1M weight elements must
land in SBUF as fp16.  Mix chosen so HBM reads, SBUF DMA writes and the
ACT/DVE/POOL cast engines all finish just under the PE floor:

  - w1 (14.7M): host-cast fp16, plain HWDGE DMA      (sync queue)
  - w3: 9 groups fp16 (8.4M), 6 groups int8 + engine-cast (scalar queue)
  - w2: 10 chunks fp16 (5.2M), 18 chunks int8 + engine-cast, staged into
    a ring during phase A via interleaved emission  (sync queue)

No SWDGE (gpsimd) DMAs: Q7 must stay free to run POOL casts - SWDGE
triggers and POOL compute share the same FIFO queue.

Phase A per pair of i-tiles (both accumulated in ONE 2KB psum bank as
[P,2,C]): t1 = p1*s1 (DVE bcast / ACT), a = Silu(t1) (ACT, batched),
t3 = p3*s3, h = t3*a (DVE/POOL).  Phase B: y^T = (w2 @ h) * s2 with 4
psum banks, scale on DVE, DMA out per m-tile.
"""

import os

import numpy as np

os.environ.setdefault("JAX_COMPILATION_CACHE_DIR", "/tmp/jax_cache")

# Problem constants (hardcoded per the task contract).
E = 8
D = 2048
I = 7168
P = 128

KD = D // P              # 16 contraction tiles for GEMM1/3
KI = I // P              # 56 i tiles
MT = D // P              # 16 output m tiles
PBM = 4                  # phase B m-tiles in flight (PSUM banks)
PBW = PBM * P            # 512: phase B weight chunk width (m cols)
MH = MT // PBM           # 4 phase-B m-groups
PBI = 8                  # phase B i-tiles per chunk
NB = KI // PBI           # 7 chunks per m-group

# phase A i-group sizes (first groups small for fast start)
GWS = [128, 384] + [512] * 13
assert sum(GWS) == I
# per-group w3 source: True -> int8 + engine-cast, False -> host fp16
W3_I8 = [False, False, False, True, False, True, False, True,
         False, True, False, True, False, True, False]
assert len(W3_I8) == len(GWS)
# per-(mh,nb) w2 source: True -> int8 + engine-cast, False -> host fp16
W2_I8 = [[(mh >= 2) or (mh == 1 and nb % 2 == 0) for nb in range(NB)]
         for mh in range(MH)]
# w2 chunks whose DMA (and cast, for int8) is emitted during phase A
PREFETCH_CHUNKS = 5

_CACHE = {}
_LAST_RESULTS = None  # for test harness introspection


def _build_nc(C):
    import contextlib

    import concourse.bacc as bacc
    import concourse.tile as tile
    from concourse import mybir

    f16 = mybir.dt.float16
    f32 = mybir.dt.float32
    i8 = mybir.dt.int8

    assert C <= 512
    jn_max = 2 if C <= 256 else 1

    nc = bacc.Bacc("TRN2", target_bir_lowering=False, debug=False, num_devices=E)

    xt = nc.dram_tensor("xt", [P, KD * C], f16, kind="ExternalInput").ap()
    # group-major partition-major weights; per group g the block is
    # [P, KD*gw] with value[p, k*gw+f] = w[gstart+f, k*P+p].  w3 ships as
    # two tensors holding the fp16 / int8 groups respectively, packed.
    w1t = nc.dram_tensor("w1t", [P, KD * I], f16, kind="ExternalInput").ap()
    n3f = KD * sum(gw for gw, i8g in zip(GWS, W3_I8) if not i8g)
    n3q = KD * sum(gw for gw, i8g in zip(GWS, W3_I8) if i8g)
    w3tf = nc.dram_tensor("w3tf", [P, n3f], f16, kind="ExternalInput").ap()
    w3tq = nc.dram_tensor("w3tq", [P, n3q], i8, kind="ExternalInput").ap()
    # phase B chunks: [P, PBI*PBW] per (mh, nb): value[p, i_l*PBW+f] =
    # w2[mh*PBW+f, (nb*PBI+i_l)*P+p]; fp16 / int8 chunks packed separately.
    n2f = PBI * PBW * sum((not q) for row in W2_I8 for q in row)
    n2q = PBI * PBW * sum(q for row in W2_I8 for q in row)
    w2tf = nc.dram_tensor("w2tf", [P, n2f], f16, kind="ExternalInput").ap()
    w2tq = nc.dram_tensor("w2tq", [P, n2q], i8, kind="ExternalInput").ap()
    s1 = nc.dram_tensor("s1", [P, KI], f32, kind="ExternalInput").ap()
    s3 = nc.dram_tensor("s3", [P, KI], f32, kind="ExternalInput").ap()
    s2 = nc.dram_tensor("s2", [P, MT], f32, kind="ExternalInput").ap()
    yt = nc.dram_tensor("yt", [D, C], f32, kind="ExternalOutput").ap()

    with tile.TileContext(nc) as tc:
        # greedy engine balancer: accumulated busy-ns per engine, costs from
        # measured rates (el/ns): cast act 44 / dve 51 / pool 34; dve fp16
        # mul ~90.  Fixed per-op overhead ~350-500ns (sem waits + dispatch).
        acc = {"act": 0.0, "dve": 0.0, "pool": 0.0}

        def pick(cost_ns, engines):
            best = min(engines, key=lambda e: acc[e] + cost_ns[e])
            acc[best] += cost_ns[best]
            return best

        def cast(out, in_, engines=("act", "dve", "pool")):
            elems = 128 * out.free_size()
            cost = {"act": elems / 44 + 400, "dve": elems / 51 + 350,
                    "pool": elems / 34 + 450}
            eng = pick(cost, engines)
            if eng == "act":
                nc.scalar.copy(out, in_)
            elif eng == "dve":
                nc.vector.tensor_copy(out, in_)
            else:
                nc.gpsimd.tensor_copy(out, in_)

        def mul(out, a, b, engines=("dve", "pool")):
            elems = 128 * out.free_size()
            cost = {"dve": elems / 90 + 350, "pool": elems / 45 + 450}
            eng = pick(cost, engines)
            if eng == "dve":
                nc.vector.tensor_mul(out, a, b)
            else:
                nc.gpsimd.tensor_mul(out, a, b)

        def psmul(out, pin, svec, i, jn):
            # out[:, j, :] = pin[:, j, :] * svec[:, i+j]; pin is PSUM so only
            # DVE (batched bcast tensor_mul) or ACT (per-j scalar.mul)
            elems = 128 * jn * out.shape[2]
            cost = {"dve": elems / 90 + 350,
                    "act": jn * (elems / jn / 44 + 400)}
            eng = pick(cost, ("dve", "act"))
            if eng == "dve":
                nc.vector.tensor_mul(
                    out, pin,
                    svec[:, i:i + jn]
                    .rearrange("p (k o) -> p k o", o=1)
                    .broadcast_to([P, jn, out.shape[2]]))
            else:
                for j in range(jn):
                    nc.scalar.mul(out[:, j, :], pin[:, j, :],
                                  svec[:, i + j:i + j + 1])

        with contextlib.ExitStack() as ctx:
            constp = ctx.enter_context(tc.tile_pool(name="const", bufs=1))
            w1p = ctx.enter_context(tc.tile_pool(name="w1p", bufs=3))
            w3sp = ctx.enter_context(tc.tile_pool(name="w3s", bufs=2))
            w3fp = ctx.enter_context(tc.tile_pool(name="w3f", bufs=2))
            hp = ctx.enter_context(tc.tile_pool(name="h", bufs=1))
            ep = ctx.enter_context(tc.tile_pool(name="eltw", bufs=2))
            w2sp = ctx.enter_context(tc.tile_pool(name="w2s", bufs=3))
            w2fp = ctx.enter_context(tc.tile_pool(name="w2f", bufs=6))
            outp = ctx.enter_context(tc.tile_pool(name="outp", bufs=4))

            # constants: x^T fp16 first on sync; scales on scalar queue
            xts = constp.tile([P, KD, C], f16)
            nc.sync.dma_start(xts, xt.rearrange("p (k c) -> p k c", c=C))
            s1s = constp.tile([P, KI], f32)
            nc.scalar.dma_start(s1s, s1)
            s3s = constp.tile([P, KI], f32)
            nc.scalar.dma_start(s3s, s3)
            s2s = constp.tile([P, MT], f32)
            nc.scalar.dma_start(s2s, s2)

            h_index = {}   # i-tile index -> (tile, j)
            GWMAX = max(GWS)
            JSZ = jn_max

            # ---- phase B chunk sources, emitted lazily so the first few
            # overlap phase A.  Returns the w2f fp16 tile for (mh, nb).
            f16off = [0]
            i8off = [0]
            w2f_tiles = {}

            def emit_w2_chunk(mh, nb):
                w2f = w2fp.tile([P, PBI, PBW], f16, tag="w2f")
                if W2_I8[mh][nb]:
                    w2s8 = w2sp.tile([P, PBI, PBW], i8, tag="w2s8")
                    nc.sync.dma_start(
                        w2s8,
                        w2tq[:, i8off[0]:i8off[0] + PBI * PBW]
                        .rearrange("p (i f) -> p i f", f=PBW))
                    i8off[0] += PBI * PBW
                    for i_l in range(0, PBI, 2):
                        cast(w2f[:, i_l:i_l + 2, :], w2s8[:, i_l:i_l + 2, :])
                else:
                    nc.sync.dma_start(
                        w2f,
                        w2tf[:, f16off[0]:f16off[0] + PBI * PBW]
                        .rearrange("p (i f) -> p i f", f=PBW))
                    f16off[0] += PBI * PBW
                w2f_tiles[(mh, nb)] = w2f

            chunk_order = [(mh, nb) for mh in range(MH) for nb in range(NB)]
            emitted = [0]

            def emit_next_chunks(n):
                for _ in range(n):
                    if emitted[0] < len(chunk_order):
                        emit_w2_chunk(*chunk_order[emitted[0]])
                        emitted[0] += 1

            # ---------------- Phase A: h = silu(x@w1^T * s1) * (x@w3^T * s3)
            f3off = 0
            q3off = 0
            with tc.tile_pool(name="psA", bufs=3, space="PSUM") as psA:
                goff = 0
                for g, gw in enumerate(GWS):
                    il0 = goff // P
                    nil = gw // P
                    w1f = w1p.tile([P, KD, GWMAX], f16, tag="w1f")
                    nc.sync.dma_start(
                        w1f[:, :, :gw],
                        w1t[:, goff * KD:(goff + gw) * KD]
                        .rearrange("p (k f) -> p k f", f=gw))
                    w3f = w3fp.tile([P, KD, GWMAX], f16, tag="w3f")
                    if W3_I8[g]:
                        w3s8 = w3sp.tile([P, KD, GWMAX], i8, tag="w3s8")
                        nc.scalar.dma_start(
                            w3s8[:, :, :gw],
                            w3tq[:, q3off:q3off + gw * KD]
                            .rearrange("p (k f) -> p k f", f=gw))
                        q3off += gw * KD
                        for k in range(0, KD, 2):
                            cast(w3f[:, k:k + 2, :gw], w3s8[:, k:k + 2, :gw])
                    else:
                        nc.scalar.dma_start(
                            w3f[:, :, :gw],
                            w3tf[:, f3off:f3off + gw * KD]
                            .rearrange("p (k f) -> p k f", f=gw))
                        f3off += gw * KD
                    # overlap part of phase B's weight traffic with phase A
                    if g >= 10 and emitted[0] < PREFETCH_CHUNKS:
                        emit_next_chunks(1)

                    il = 0
                    while il < nil:
                        jn = min(jn_max, nil - il)
                        i = il0 + il
                        p1 = psA.tile([P, JSZ, C], f32, tag="p1")
                        p3 = psA.tile([P, JSZ, C], f32, tag="p3")
                        for j in range(jn):
                            lo = (il + j) * P
                            for k in range(KD):
                                nc.tensor.matmul(
                                    p1[:, j, :], w1f[:, k, lo:lo + P],
                                    xts[:, k, :],
                                    start=(k == 0), stop=(k == KD - 1))
                        for j in range(jn):
                            lo = (il + j) * P
                            for k in range(KD):
                                nc.tensor.matmul(
                                    p3[:, j, :], w3f[:, k, lo:lo + P],
                                    xts[:, k, :],
                                    start=(k == 0), stop=(k == KD - 1))
                        t1 = ep.tile([P, JSZ, C], f16, tag="t1")
                        psmul(t1[:, :jn, :], p1[:, :jn, :], s1s, i, jn)
                        a = ep.tile([P, JSZ, C], f16, tag="a")
                        nc.scalar.activation(
                            a[:, :jn, :], t1[:, :jn, :],
                            mybir.ActivationFunctionType.Silu)
                        acc["act"] += 128 * jn * C / 55 + 400
                        t3 = ep.tile([P, JSZ, C], f16, tag="t3")
                        psmul(t3[:, :jn, :], p3[:, :jn, :], s3s, i, jn)
                        htile = hp.tile([P, JSZ, C], f16, tag=f"h{i}")
                        mul(htile[:, :jn, :], t3[:, :jn, :], a[:, :jn, :])
                        for j in range(jn):
                            h_index[i + j] = (htile, j)
                        il += jn
                    goff += gw

            # ---------------- Phase B: y^T = (w2 @ h) * s2
            with tc.tile_pool(name="psB", bufs=1, space="PSUM") as psB:
                for mh in range(MH):
                    pbs = [psB.tile([P, C], f32, tag=f"pb{ml}",
                                    name=f"pb{mh}_{ml}")
                           for ml in range(PBM)]
                    for nb in range(NB):
                        if (mh, nb) not in w2f_tiles:
                            emit_next_chunks(1 + (emitted[0] < 12))
                        w2f = w2f_tiles[(mh, nb)]
                        for i_l in range(PBI):
                            i = nb * PBI + i_l
                            ht, j = h_index[i]
                            for ml in range(PBM):
                                nc.tensor.matmul(
                                    pbs[ml],
                                    w2f[:, i_l, ml * P:(ml + 1) * P],
                                    ht[:, j, :],
                                    start=(i == 0), stop=(i == KI - 1))
                    for ml in range(PBM):
                        m = mh * PBM + ml
                        o = outp.tile([P, C], f32, tag="o")
                        nc.vector.tensor_scalar_mul(o, pbs[ml], s2s[:, m:m + 1])
                        nc.sync.dma_start(yt[m * P:(m + 1) * P, :], o)

    nc.compile()
    return nc


def _tile_w13(w, dtype_sel):
    """[I, D] -> (fp16 blocks, int8 blocks) packed per GWS/dtype_sel."""
    fparts, qparts = [], []
    goff = 0
    for gw, is_i8 in zip(GWS, dtype_sel):
        blk = w[goff:goff + gw, :]                      # [gw, D] int8
        t = blk.reshape(gw, KD, P).transpose(2, 1, 0).reshape(P, KD * gw)
        if is_i8:
            qparts.append(t)
        else:
            fparts.append(t.astype(np.float16))
        goff += gw
    fcat = (np.ascontiguousarray(np.concatenate(fparts, axis=1))
            if fparts else np.zeros((P, 0), np.float16))
    qcat = (np.ascontiguousarray(np.concatenate(qparts, axis=1))
            if qparts else np.zeros((P, 0), np.int8))
    return fcat, qcat


def _tile_w2(w):
    """[D, I] int8 -> (fp16 chunks, int8 chunks) packed per W2_I8."""
    t = w.reshape(MH, PBW, KI, P).transpose(0, 3, 2, 1)  # [MH, P, KI, PBW]
    fparts, qparts = [], []
    for mh in range(MH):
        for nb in range(NB):
            c = t[mh, :, nb * PBI:(nb + 1) * PBI, :].reshape(P, PBI * PBW)
            if W2_I8[mh][nb]:
                qparts.append(c)
            else:
                fparts.append(c.astype(np.float16))
    fcat = (np.ascontiguousarray(np.concatenate(fparts, axis=1))
            if fparts else np.zeros((P, 0), np.float16))
    qcat = (np.ascontiguousarray(np.concatenate(qparts, axis=1))
            if qparts else np.zeros((P, 0), np.int8))
    return fcat, qcat


def _route(expert_indices):
    idx = np.asarray(expert_indices).astype(np.int64)
    toks, slots = [], []
    for e in range(E):
        t, a = np.nonzero(idx == e)
        toks.append(t)
        slots.append(a)
    return toks, slots


def _prepare(inputs):
    x = np.asarray(inputs["x"], dtype=np.float32)          # [T, D]
    expert_indices = np.asarray(inputs["expert_indices"])  # [T, A]
    w1 = np.asarray(inputs["w1"])                          # [E, I, D] int8
    w2 = np.asarray(inputs["w2"])                          # [E, D, I] int8
    w3 = np.asarray(inputs["w3"])                          # [E, I, D] int8
    scales1 = np.asarray(inputs["scales1"], dtype=np.float32)
    scales2 = np.asarray(inputs["scales2"], dtype=np.float32)
    scales3 = np.asarray(inputs["scales3"], dtype=np.float32)

    T, A = expert_indices.shape
    toks, slots = _route(expert_indices)
    counts = [len(t) for t in toks]
    C = max(max(counts), 8)
    C = (C + 1) // 2 * 2

    if C not in _CACHE:
        _CACHE[C] = _build_nc(C)
    nc = _CACHE[C]

    in_maps = []
    for e in range(E):
        n_e = counts[e]
        xtc = np.zeros((P, KD, C), dtype=np.float16)
        if n_e:
            xe = x[toks[e]].astype(np.float16)             # [n_e, D]
            xtc[:, :, :n_e] = xe.T.reshape(KD, P, n_e).transpose(1, 0, 2)
        w1f, _ = _tile_w13(w1[e], [False] * len(GWS))
        w3f, w3q = _tile_w13(w3[e], W3_I8)
        w2f, w2q = _tile_w2(w2[e])
        in_maps.append(
            dict(
                xt=np.ascontiguousarray(xtc.reshape(P, KD * C)),
                w1t=w1f,
                w3tf=w3f,
                w3tq=w3q,
                w2tf=w2f,
                w2tq=w2q,
                s1=np.ascontiguousarray(scales1[e].reshape(KI, P).T),
                s3=np.ascontiguousarray(scales3[e].reshape(KI, P).T),
                s2=np.ascontiguousarray(scales2[e].reshape(MT, P).T),
            )
        )
    return nc, in_maps, (T, A, toks, slots, counts)


def kernel(**inputs):
    global _LAST_RESULTS
    from concourse.bass_utils import run_bass_kernel_spmd

    nc, in_maps, (T, A, toks, slots, counts) = _prepare(inputs)
    res = run_bass_kernel_spmd(nc, in_maps, core_ids=list(range(E)))
    _LAST_RESULTS = res

    out = np.zeros((T, A, D), dtype=np.float32)
    for e in range(E):
        n_e = counts[e]
        if n_e:
            ye = res.results[e]["yt"][:, :n_e].T  # [n_e, D]
            out[toks[e], slots[e], :] = ye
    return out


# revision 9
# speedup vs baseline: 1.1516x; 1.0944x over previous
"""MoE ConditionalFeedForward (int8 SwiGLU experts) on 8 trn2 NeuronCores.

Expert-parallel: host routes token(+slot) pairs to their expert, pads each
expert's batch to a common capacity C, pre-tiles the weights into the exact
contiguous chunks the kernel DMAs, and ships one expert per core.

The PE matmul floor is 2688 MMs x (C/2.4+2.5)ns ~= 177us at C=152; the
binding resource around it is weight ingest: 44.1M weight elements must
land in SBUF as fp16.  Mix chosen so HBM reads, SBUF DMA writes and the
ACT/DVE/POOL cast engines all finish just under the PE floor:

  - w1 (14.7M): host-cast fp16, plain HWDGE DMA      (sync queue)
  - w3: 9 groups fp16 (8.4M), 6 groups int8 + engine-cast (scalar queue)
  - w2: 10 chunks fp16 (5.2M), 18 chunks int8 + engine-cast, staged into
    a ring during phase A via interleaved emission  (sync queue)

No SWDGE (gpsimd) DMAs: Q7 must stay free to run POOL casts - SWDGE
triggers and POOL compute share the same FIFO queue.

Phase A per pair of i-tiles (both accumulated in ONE 2KB psum bank as
[P,2,C]): t1 = p1*s1 (DVE bcast / ACT), a = Silu(t1) (ACT, batched),
t3 = p3*s3, h = t3*a (DVE/POOL).  Phase B: y^T = (w2 @ h) * s2 with 4
psum banks, scale on DVE, DMA out per m-tile.
"""

import os

import numpy as np

os.environ.setdefault("JAX_COMPILATION_CACHE_DIR", "/tmp/jax_cache")

# Problem constants (hardcoded per the task contract).
E = 8
D = 2048
I = 7168
P = 128

KD = D // P              # 16 contraction tiles for GEMM1/3
KI = I // P              # 56 i tiles
MT = D // P              # 16 output m tiles
PBM = 4                  # phase B m-tiles in flight (PSUM banks)
PBW = PBM * P            # 512: phase B weight chunk width (m cols)
MH = MT // PBM           # 4 phase-B m-groups
PBI = 8                  # phase B i-tiles per chunk
NB = KI // PBI           # 7 chunks per m-group

# phase A i-group sizes (first groups small for fast start)
GWS = [128, 384] + [512] * 13
assert sum(GWS) == I
# per-group w3 source: True -> int8 + engine-cast, False -> host fp16
W3_I8 = [False, False, False, True, False, True, False, True,
         False, True, False, True, False, True, False]
assert len(W3_I8) == len(GWS)
# per-(mh,nb) w2 source: True -> int8 + engine-cast, False -> host fp16
W2_I8 = [[(mh >= 2) or (mh == 1 and nb % 2 == 0) for nb in range(NB)]
         for mh in range(MH)]
# w2 chunks whose DMA (and cast, for int8) is emitted during phase A
PREFETCH_CHUNKS = 4

_CACHE = {}
_LAST_RESULTS = None  # for test harness introspection


def _build_nc(C):
    import contextlib

    import concourse.bacc as bacc
    import concourse.tile as tile
    from concourse import mybir

    f16 = mybir.dt.float16
    f32 = mybir.dt.float32
    i8 = mybir.dt.int8

    assert C <= 512
    jn_max = 2 if C <= 256 else 1

    nc = bacc.Bacc("TRN2", target_bir_lowering=False, debug=False, num_devices=E)

    xt = nc.dram_tensor("xt", [P, KD * C], f16, kind="ExternalInput").ap()
    # group-major partition-major weights; per group g the block is
    # [P, KD*gw] with value[p, k*gw+f] = w[gstart+f, k*P+p].  w3 ships as
    # two tensors holding the fp16 / int8 groups respectively, packed.
    w1t = nc.dram_tensor("w1t", [P, KD * I], f16, kind="ExternalInput").ap()
    n3f = KD * sum(gw for gw, i8g in zip(GWS, W3_I8) if not i8g)
    n3q = KD * sum(gw for gw, i8g in zip(GWS, W3_I8) if i8g)
    w3tf = nc.dram_tensor("w3tf", [P, n3f], f16, kind="ExternalInput").ap()
    w3tq = nc.dram_tensor("w3tq", [P, n3q], i8, kind="ExternalInput").ap()
    # phase B chunks: [P, PBI*PBW] per (mh, nb): value[p, i_l*PBW+f] =
    # w2[mh*PBW+f, (nb*PBI+i_l)*P+p]; fp16 / int8 chunks packed separately.
    n2f = PBI * PBW * sum((not q) for row in W2_I8 for q in row)
    n2q = PBI * PBW * sum(q for row in W2_I8 for q in row)
    w2tf = nc.dram_tensor("w2tf", [P, n2f], f16, kind="ExternalInput").ap()
    w2tq = nc.dram_tensor("w2tq", [P, n2q], i8, kind="ExternalInput").ap()
    s1 = nc.dram_tensor("s1", [P, KI], f32, kind="ExternalInput").ap()
    s3 = nc.dram_tensor("s3", [P, KI], f32, kind="ExternalInput").ap()
    s2 = nc.dram_tensor("s2", [P, MT], f32, kind="ExternalInput").ap()
    yt = nc.dram_tensor("yt", [D, C], f32, kind="ExternalOutput").ap()

    with tile.TileContext(nc) as tc:
        # greedy engine balancer: accumulated busy-ns per engine, costs from
        # measured rates (el/ns): cast act 44 / dve 51 / pool 34; dve fp16
        # mul ~90.  Fixed per-op overhead ~350-500ns (sem waits + dispatch).
        acc = {"act": 0.0, "dve": 0.0, "pool": 0.0}

        def pick(cost_ns, engines):
            best = min(engines, key=lambda e: acc[e] + cost_ns[e])
            acc[best] += cost_ns[best]
            return best

        def cast(out, in_, engines=("act", "dve", "pool")):
            elems = 128 * out.free_size()
            cost = {"act": elems / 44 + 400, "dve": elems / 51 + 350,
                    "pool": elems / 34 + 450}
            eng = pick(cost, engines)
            if eng == "act":
                nc.scalar.copy(out, in_)
            elif eng == "dve":
                nc.vector.tensor_copy(out, in_)
            else:
                nc.gpsimd.tensor_copy(out, in_)

        def mul(out, a, b, engines=("dve", "pool")):
            elems = 128 * out.free_size()
            cost = {"dve": elems / 90 + 350, "pool": elems / 45 + 450}
            eng = pick(cost, engines)
            if eng == "dve":
                nc.vector.tensor_mul(out, a, b)
            else:
                nc.gpsimd.tensor_mul(out, a, b)

        def psmul(out, pin, svec, i, jn):
            # out[:, j, :] = pin[:, j, :] * svec[:, i+j]; pin is PSUM so only
            # DVE (batched bcast tensor_mul) or ACT (per-j scalar.mul)
            elems = 128 * jn * out.shape[2]
            cost = {"dve": elems / 90 + 350,
                    "act": jn * (elems / jn / 44 + 400)}
            eng = pick(cost, ("dve", "act"))
            if eng == "dve":
                nc.vector.tensor_mul(
                    out, pin,
                    svec[:, i:i + jn]
                    .rearrange("p (k o) -> p k o", o=1)
                    .broadcast_to([P, jn, out.shape[2]]))
            else:
                for j in range(jn):
                    nc.scalar.mul(out[:, j, :], pin[:, j, :],
                                  svec[:, i + j:i + j + 1])

        with contextlib.ExitStack() as ctx:
            constp = ctx.enter_context(tc.tile_pool(name="const", bufs=1))
            w1p = ctx.enter_context(tc.tile_pool(name="w1p", bufs=3))
            w3sp = ctx.enter_context(tc.tile_pool(name="w3s", bufs=3))
            w3fp = ctx.enter_context(tc.tile_pool(name="w3f", bufs=3))
            hp = ctx.enter_context(tc.tile_pool(name="h", bufs=1))
            ep = ctx.enter_context(tc.tile_pool(name="eltw", bufs=2))
            w2sp = ctx.enter_context(tc.tile_pool(name="w2s", bufs=3))
            w2fp = ctx.enter_context(tc.tile_pool(name="w2f", bufs=5))
            outp = ctx.enter_context(tc.tile_pool(name="outp", bufs=4))

            # constants: x^T fp16 first on sync; scales on scalar queue
            xts = constp.tile([P, KD, C], f16)
            nc.sync.dma_start(xts, xt.rearrange("p (k c) -> p k c", c=C))
            s1s = constp.tile([P, KI], f32)
            nc.scalar.dma_start(s1s, s1)
            s3s = constp.tile([P, KI], f32)
            nc.scalar.dma_start(s3s, s3)
            s2s = constp.tile([P, MT], f32)
            nc.scalar.dma_start(s2s, s2)

            h_index = {}   # i-tile index -> (tile, j)
            GWMAX = max(GWS)
            JSZ = jn_max

            # ---- phase B chunk sources: DMA and cast emission are split so
            # both can be emitted ahead of the consuming matmuls (the engine
            # queues are strict FIFO - an op waiting on a psum drain would
            # head-of-line-block casts emitted after it).
            f16off = [0]
            i8off = [0]
            w2f_tiles = {}
            w2s_tiles = {}
            NCH = MH * NB

            def emit_w2_dma(c):
                if c >= NCH or c in w2f_tiles:
                    return
                mh, nb = divmod(c, NB)
                w2f = w2fp.tile([P, PBI, PBW], f16, tag="w2f")
                if W2_I8[mh][nb]:
                    w2s8 = w2sp.tile([P, PBI, PBW], i8, tag="w2s8")
                    nc.sync.dma_start(
                        w2s8,
                        w2tq[:, i8off[0]:i8off[0] + PBI * PBW]
                        .rearrange("p (i f) -> p i f", f=PBW))
                    i8off[0] += PBI * PBW
                    w2s_tiles[c] = w2s8
                else:
                    nc.sync.dma_start(
                        w2f,
                        w2tf[:, f16off[0]:f16off[0] + PBI * PBW]
                        .rearrange("p (i f) -> p i f", f=PBW))
                    f16off[0] += PBI * PBW
                w2f_tiles[c] = w2f

            w2_casted = set()

            def emit_w2_cast(c):
                if c >= NCH or c in w2_casted:
                    return
                w2_casted.add(c)
                if c in w2s_tiles:
                    w2s8 = w2s_tiles[c]
                    w2f = w2f_tiles[c]
                    for i_l in range(0, PBI, 2):
                        cast(w2f[:, i_l:i_l + 2, :], w2s8[:, i_l:i_l + 2, :])

            # ---------------- Phase A: h = silu(x@w1^T * s1) * (x@w3^T * s3)
            # software-pipelined emission: DMA group g+2, cast group g+1,
            # then compute group g, so casts never wait behind chain ops.
            f3off = [0]
            q3off = [0]
            w1f_t = {}
            w3f_t = {}
            w3s_t = {}
            w3_casted = set()

            def emit_w13_dma(g):
                if g >= len(GWS) or g in w1f_t:
                    return
                gw = GWS[g]
                goff = sum(GWS[:g])
                w1f = w1p.tile([P, KD, GWMAX], f16, tag="w1f")
                nc.sync.dma_start(
                    w1f[:, :, :gw],
                    w1t[:, goff * KD:(goff + gw) * KD]
                    .rearrange("p (k f) -> p k f", f=gw))
                w1f_t[g] = w1f
                w3f = w3fp.tile([P, KD, GWMAX], f16, tag="w3f")
                if W3_I8[g]:
                    w3s8 = w3sp.tile([P, KD, GWMAX], i8, tag="w3s8")
                    nc.scalar.dma_start(
                        w3s8[:, :, :gw],
                        w3tq[:, q3off[0]:q3off[0] + gw * KD]
                        .rearrange("p (k f) -> p k f", f=gw))
                    q3off[0] += gw * KD
                    w3s_t[g] = w3s8
                else:
                    nc.scalar.dma_start(
                        w3f[:, :, :gw],
                        w3tf[:, f3off[0]:f3off[0] + gw * KD]
                        .rearrange("p (k f) -> p k f", f=gw))
                    f3off[0] += gw * KD
                w3f_t[g] = w3f

            def emit_w3_cast(g):
                if g >= len(GWS) or g in w3_casted:
                    return
                w3_casted.add(g)
                if g in w3s_t:
                    gw = GWS[g]
                    for k in range(0, KD, 2):
                        cast(w3f_t[g][:, k:k + 2, :gw],
                             w3s_t[g][:, k:k + 2, :gw])

            with tc.tile_pool(name="psA", bufs=4, space="PSUM") as psA:
                emit_w13_dma(0)
                emit_w13_dma(1)
                emit_w3_cast(0)
                goff = 0
                for g, gw in enumerate(GWS):
                    il0 = goff // P
                    nil = gw // P
                    emit_w13_dma(g + 2)
                    emit_w3_cast(g + 1)
                    # overlap part of phase B's weight traffic with phase A
                    if g >= 10 and len(w2f_tiles) < PREFETCH_CHUNKS:
                        emit_w2_dma(len(w2f_tiles))
                    w1f = w1f_t[g]
                    w3f = w3f_t[g]

                    il = 0
                    while il < nil:
                        jn = min(jn_max, nil - il)
                        i = il0 + il
                        p1 = psA.tile([P, JSZ, C], f32, tag="p1")
                        p3 = psA.tile([P, JSZ, C], f32, tag="p3")
                        for j in range(jn):
                            lo = (il + j) * P
                            for k in range(KD):
                                nc.tensor.matmul(
                                    p1[:, j, :], w1f[:, k, lo:lo + P],
                                    xts[:, k, :],
                                    start=(k == 0), stop=(k == KD - 1))
                        for j in range(jn):
                            lo = (il + j) * P
                            for k in range(KD):
                                nc.tensor.matmul(
                                    p3[:, j, :], w3f[:, k, lo:lo + P],
                                    xts[:, k, :],
                                    start=(k == 0), stop=(k == KD - 1))
                        t1 = ep.tile([P, JSZ, C], f16, tag="t1")
                        psmul(t1[:, :jn, :], p1[:, :jn, :], s1s, i, jn)
                        a = ep.tile([P, JSZ, C], f16, tag="a")
                        nc.scalar.activation(
                            a[:, :jn, :], t1[:, :jn, :],
                            mybir.ActivationFunctionType.Silu)
                        acc["act"] += 128 * jn * C / 55 + 400
                        t3 = ep.tile([P, JSZ, C], f16, tag="t3")
                        psmul(t3[:, :jn, :], p3[:, :jn, :], s3s, i, jn)
                        htile = hp.tile([P, JSZ, C], f16, tag=f"h{i}")
                        mul(htile[:, :jn, :], t3[:, :jn, :], a[:, :jn, :])
                        for j in range(jn):
                            h_index[i + j] = (htile, j)
                        il += jn
                    goff += gw

            # ---------------- Phase B: y^T = (w2 @ h) * s2
            with tc.tile_pool(name="psB", bufs=2, space="PSUM") as psB:
                for mh in range(MH):
                    pbs = [psB.tile([P, C], f32, tag=f"pb{ml}",
                                    name=f"pb{mh}_{ml}")
                           for ml in range(PBM)]
                    for nb in range(NB):
                        c = mh * NB + nb
                        emit_w2_dma(c)       # no-op unless pipeline fell behind
                        emit_w2_cast(c)
                        emit_w2_dma(c + 2)
                        emit_w2_cast(c + 1)
                        w2f = w2f_tiles[c]
                        for i_l in range(PBI):
                            i = nb * PBI + i_l
                            ht, j = h_index[i]
                            for ml in range(PBM):
                                nc.tensor.matmul(
                                    pbs[ml],
                                    w2f[:, i_l, ml * P:(ml + 1) * P],
                                    ht[:, j, :],
                                    start=(i == 0), stop=(i == KI - 1))
                    for ml in range(PBM):
                        m = mh * PBM + ml
                        o = outp.tile([P, C], f32, tag="o")
                        nc.vector.tensor_scalar_mul(o, pbs[ml], s2s[:, m:m + 1])
                        q = nc.sync if ml % 2 == 0 else nc.scalar
                        q.dma_start(yt[m * P:(m + 1) * P, :], o)

    nc.compile()
    return nc


def _tile_w13(w, dtype_sel):
    """[I, D] -> (fp16 blocks, int8 blocks) packed per GWS/dtype_sel."""
    fparts, qparts = [], []
    goff = 0
    for gw, is_i8 in zip(GWS, dtype_sel):
        blk = w[goff:goff + gw, :]                      # [gw, D] int8
        t = blk.reshape(gw, KD, P).transpose(2, 1, 0).reshape(P, KD * gw)
        if is_i8:
            qparts.append(t)
        else:
            fparts.append(t.astype(np.float16))
        goff += gw
    fcat = (np.ascontiguousarray(np.concatenate(fparts, axis=1))
            if fparts else np.zeros((P, 0), np.float16))
    qcat = (np.ascontiguousarray(np.concatenate(qparts, axis=1))
            if qparts else np.zeros((P, 0), np.int8))
    return fcat, qcat


def _tile_w2(w):
    """[D, I] int8 -> (fp16 chunks, int8 chunks) packed per W2_I8."""
    t = w.reshape(MH, PBW, KI, P).transpose(0, 3, 2, 1)  # [MH, P, KI, PBW]
    fparts, qparts = [], []
    for mh in range(MH):
        for nb in range(NB):
            c = t[mh, :, nb * PBI:(nb + 1) * PBI, :].reshape(P, PBI * PBW)
            if W2_I8[mh][nb]:
                qparts.append(c)
            else:
                fparts.append(c.astype(np.float16))
    fcat = (np.ascontiguousarray(np.concatenate(fparts, axis=1))
            if fparts else np.zeros((P, 0), np.float16))
    qcat = (np.ascontiguousarray(np.concatenate(qparts, axis=1))
            if qparts else np.zeros((P, 0), np.int8))
    return fcat, qcat


def _route(expert_indices):
    idx = np.asarray(expert_indices).astype(np.int64)
    toks, slots = [], []
    for e in range(E):
        t, a = np.nonzero(idx == e)
        toks.append(t)
        slots.append(a)
    return toks, slots


def _prepare(inputs):
    x = np.asarray(inputs["x"], dtype=np.float32)          # [T, D]
    expert_indices = np.asarray(inputs["expert_indices"])  # [T, A]
    w1 = np.asarray(inputs["w1"])                          # [E, I, D] int8
    w2 = np.asarray(inputs["w2"])                          # [E, D, I] int8
    w3 = np.asarray(inputs["w3"])                          # [E, I, D] int8
    scales1 = np.asarray(inputs["scales1"], dtype=np.float32)
    scales2 = np.asarray(inputs["scales2"], dtype=np.float32)
    scales3 = np.asarray(inputs["scales3"], dtype=np.float32)

    T, A = expert_indices.shape
    toks, slots = _route(expert_indices)
    counts = [len(t) for t in toks]
    C = max(max(counts), 8)
    C = (C + 1) // 2 * 2

    if C not in _CACHE:
        _CACHE[C] = _build_nc(C)
    nc = _CACHE[C]

    in_maps = []
    for e in range(E):
        n_e = counts[e]
        xtc = np.zeros((P, KD, C), dtype=np.float16)
        if n_e:
            xe = x[toks[e]].astype(np.float16)             # [n_e, D]
            xtc[:, :, :n_e] = xe.T.reshape(KD, P, n_e).transpose(1, 0, 2)
        w1f, _ = _tile_w13(w1[e], [False] * len(GWS))
        w3f, w3q = _tile_w13(w3[e], W3_I8)
        w2f, w2q = _tile_w2(w2[e])
        in_maps.append(
            dict(
                xt=np.ascontiguousarray(xtc.reshape(P, KD * C)),
                w1t=w1f,
                w3tf=w3f,
                w3tq=w3q,
                w2tf=w2f,
                w2tq=w2q,
                s1=np.ascontiguousarray(scales1[e].reshape(KI, P).T),
                s3=np.ascontiguousarray(scales3[e].reshape(KI, P).T),
                s2=np.ascontiguousarray(scales2[e].reshape(MT, P).T),
            )
        )
    return nc, in_maps, (T, A, toks, slots, counts)


def kernel(**inputs):
    global _LAST_RESULTS
    from concourse.bass_utils import run_bass_kernel_spmd

    nc, in_maps, (T, A, toks, slots, counts) = _prepare(inputs)
    res = run_bass_kernel_spmd(nc, in_maps, core_ids=list(range(E)))
    _LAST_RESULTS = res

    out = np.zeros((T, A, D), dtype=np.float32)
    for e in range(E):
        n_e = counts[e]
        if n_e:
            ye = res.results[e]["yt"][:, :n_e].T  # [n_e, D]
            out[toks[e], slots[e], :] = ye
    return out


# revision 10
# speedup vs baseline: 1.1646x; 1.0114x over previous
"""MoE ConditionalFeedForward (int8 SwiGLU experts) on 8 trn2 NeuronCores.

Expert-parallel: host routes token(+slot) pairs to their expert, pads each
expert's batch to a common capacity C, pre-tiles the weights into the exact
contiguous chunks the kernel DMAs, and ships one expert per core.

The PE matmul floor is 2688 MMs x (C/2.4+2.5)ns ~= 177us at C=152; the
binding resource around it is weight ingest: 44.1M weight elements must
land in SBUF as fp16.  Mix chosen so HBM reads, SBUF DMA writes and the
ACT/DVE/POOL cast engines all finish just under the PE floor:

  - w1 (14.7M): host-cast fp16, plain HWDGE DMA      (sync queue)
  - w3: 9 groups fp16 (8.4M), 6 groups int8 + engine-cast (scalar queue)
  - w2: 10 chunks fp16 (5.2M), 18 chunks int8 + engine-cast, staged into
    a ring during phase A via interleaved emission  (sync queue)

No SWDGE (gpsimd) DMAs: Q7 must stay free to run POOL casts - SWDGE
triggers and POOL compute share the same FIFO queue.

Phase A per pair of i-tiles (both accumulated in ONE 2KB psum bank as
[P,2,C]): t1 = p1*s1 (DVE bcast / ACT), a = Silu(t1) (ACT, batched),
t3 = p3*s3, h = t3*a (DVE/POOL).  Phase B: y^T = (w2 @ h) * s2 with 4
psum banks, scale on DVE, DMA out per m-tile.
"""

import os

import numpy as np

os.environ.setdefault("JAX_COMPILATION_CACHE_DIR", "/tmp/jax_cache")

# Problem constants (hardcoded per the task contract).
E = 8
D = 2048
I = 7168
P = 128

KD = D // P              # 16 contraction tiles for GEMM1/3
KI = I // P              # 56 i tiles
MT = D // P              # 16 output m tiles
PBM = 4                  # phase B m-tiles in flight (PSUM banks)
PBW = PBM * P            # 512: phase B weight chunk width (m cols)
MH = MT // PBM           # 4 phase-B m-groups
PBI = 8                  # phase B i-tiles per chunk
NB = KI // PBI           # 7 chunks per m-group

# phase A i-group sizes (first groups small for fast start)
GWS = [128, 384] + [512] * 13
assert sum(GWS) == I
# per-group w3 source: True -> int8 + engine-cast, False -> host fp16
W3_I8 = [True, True, True, True, False, True, False, True,
         False, True, False, False, True, False, False]
assert len(W3_I8) == len(GWS)
# per-(mh,nb) w2 source: True -> int8 + engine-cast, False -> host fp16
W2_I8 = [[(mh >= 1) and (nb % 3 != 2) for nb in range(NB)]
         for mh in range(MH)]
# w2 chunks whose DMA (and cast, for int8) is emitted during phase A
PREFETCH_CHUNKS = 4

_CACHE = {}
_LAST_RESULTS = None  # for test harness introspection


def _build_nc(C):
    import contextlib

    import concourse.bacc as bacc
    import concourse.tile as tile
    from concourse import mybir

    f16 = mybir.dt.float16
    f32 = mybir.dt.float32
    i8 = mybir.dt.int8

    assert C <= 512
    jn_max = 2 if C <= 256 else 1

    nc = bacc.Bacc("TRN2", target_bir_lowering=False, debug=False, num_devices=E)

    xt = nc.dram_tensor("xt", [P, KD * C], f16, kind="ExternalInput").ap()
    # group-major partition-major weights; per group g the block is
    # [P, KD*gw] with value[p, k*gw+f] = w[gstart+f, k*P+p].  w3 ships as
    # two tensors holding the fp16 / int8 groups respectively, packed.
    w1t = nc.dram_tensor("w1t", [P, KD * I], f16, kind="ExternalInput").ap()
    n3f = KD * sum(gw for gw, i8g in zip(GWS, W3_I8) if not i8g)
    n3q = KD * sum(gw for gw, i8g in zip(GWS, W3_I8) if i8g)
    w3tf = nc.dram_tensor("w3tf", [P, n3f], f16, kind="ExternalInput").ap()
    w3tq = nc.dram_tensor("w3tq", [P, n3q], i8, kind="ExternalInput").ap()
    # phase B chunks: [P, PBI*PBW] per (mh, nb): value[p, i_l*PBW+f] =
    # w2[mh*PBW+f, (nb*PBI+i_l)*P+p]; fp16 / int8 chunks packed separately.
    n2f = PBI * PBW * sum((not q) for row in W2_I8 for q in row)
    n2q = PBI * PBW * sum(q for row in W2_I8 for q in row)
    w2tf = nc.dram_tensor("w2tf", [P, n2f], f16, kind="ExternalInput").ap()
    w2tq = nc.dram_tensor("w2tq", [P, n2q], i8, kind="ExternalInput").ap()
    s1 = nc.dram_tensor("s1", [P, KI], f32, kind="ExternalInput").ap()
    s3 = nc.dram_tensor("s3", [P, KI], f32, kind="ExternalInput").ap()
    s2 = nc.dram_tensor("s2", [P, MT], f32, kind="ExternalInput").ap()
    yt = nc.dram_tensor("yt", [D, C], f32, kind="ExternalOutput").ap()

    with tile.TileContext(nc) as tc:
        # greedy engine balancer: accumulated busy-ns per engine, costs from
        # measured rates (el/ns): cast act 44 / dve 51 / pool 34; dve fp16
        # mul ~90.  Fixed per-op overhead ~350-500ns (sem waits + dispatch).
        acc = {"act": 0.0, "dve": 0.0, "pool": 0.0}

        def pick(cost_ns, engines):
            best = min(engines, key=lambda e: acc[e] + cost_ns[e])
            acc[best] += cost_ns[best]
            return best

        def cast(out, in_, engines=("act", "dve", "pool")):
            elems = 128 * out.free_size()
            cost = {"act": elems / 44 + 400, "dve": elems / 51 + 350,
                    "pool": elems / 34 + 450}
            eng = pick(cost, engines)
            if eng == "act":
                nc.scalar.copy(out, in_)
            elif eng == "dve":
                nc.vector.tensor_copy(out, in_)
            else:
                nc.gpsimd.tensor_copy(out, in_)

        def mul(out, a, b, engines=("dve", "pool")):
            elems = 128 * out.free_size()
            cost = {"dve": elems / 90 + 350, "pool": elems / 45 + 450}
            eng = pick(cost, engines)
            if eng == "dve":
                nc.vector.tensor_mul(out, a, b)
            else:
                nc.gpsimd.tensor_mul(out, a, b)

        def psmul(out, pin, svec, i, jn):
            # out[:, j, :] = pin[:, j, :] * svec[:, i+j]; pin is PSUM so only
            # DVE (batched bcast tensor_mul) or ACT (per-j scalar.mul)
            elems = 128 * jn * out.shape[2]
            cost = {"dve": elems / 90 + 350,
                    "act": jn * (elems / jn / 44 + 400)}
            eng = pick(cost, ("dve", "act"))
            if eng == "dve":
                nc.vector.tensor_mul(
                    out, pin,
                    svec[:, i:i + jn]
                    .rearrange("p (k o) -> p k o", o=1)
                    .broadcast_to([P, jn, out.shape[2]]))
            else:
                for j in range(jn):
                    nc.scalar.mul(out[:, j, :], pin[:, j, :],
                                  svec[:, i + j:i + j + 1])

        with contextlib.ExitStack() as ctx:
            constp = ctx.enter_context(tc.tile_pool(name="const", bufs=1))
            w1p = ctx.enter_context(tc.tile_pool(name="w1p", bufs=3))
            w3sp = ctx.enter_context(tc.tile_pool(name="w3s", bufs=3))
            w3fp = ctx.enter_context(tc.tile_pool(name="w3f", bufs=3))
            hp = ctx.enter_context(tc.tile_pool(name="h", bufs=1))
            ep = ctx.enter_context(tc.tile_pool(name="eltw", bufs=2))
            w2sp = ctx.enter_context(tc.tile_pool(name="w2s", bufs=3))
            w2fp = ctx.enter_context(tc.tile_pool(name="w2f", bufs=5))
            outp = ctx.enter_context(tc.tile_pool(name="outp", bufs=4))

            # constants: x^T fp16 first on sync; scales on scalar queue
            xts = constp.tile([P, KD, C], f16)
            nc.sync.dma_start(xts, xt.rearrange("p (k c) -> p k c", c=C))
            s1s = constp.tile([P, KI], f32)
            nc.scalar.dma_start(s1s, s1)
            s3s = constp.tile([P, KI], f32)
            nc.scalar.dma_start(s3s, s3)
            s2s = constp.tile([P, MT], f32)
            nc.scalar.dma_start(s2s, s2)

            h_index = {}   # i-tile index -> (tile, j)
            GWMAX = max(GWS)
            JSZ = jn_max

            # ---- phase B chunk sources: DMA and cast emission are split so
            # both can be emitted ahead of the consuming matmuls (the engine
            # queues are strict FIFO - an op waiting on a psum drain would
            # head-of-line-block casts emitted after it).
            f16off = [0]
            i8off = [0]
            w2f_tiles = {}
            w2s_tiles = {}
            NCH = MH * NB

            def emit_w2_dma(c):
                if c >= NCH or c in w2f_tiles:
                    return
                mh, nb = divmod(c, NB)
                w2f = w2fp.tile([P, PBI, PBW], f16, tag="w2f")
                if W2_I8[mh][nb]:
                    w2s8 = w2sp.tile([P, PBI, PBW], i8, tag="w2s8")
                    nc.sync.dma_start(
                        w2s8,
                        w2tq[:, i8off[0]:i8off[0] + PBI * PBW]
                        .rearrange("p (i f) -> p i f", f=PBW))
                    i8off[0] += PBI * PBW
                    w2s_tiles[c] = w2s8
                else:
                    nc.scalar.dma_start(
                        w2f,
                        w2tf[:, f16off[0]:f16off[0] + PBI * PBW]
                        .rearrange("p (i f) -> p i f", f=PBW))
                    f16off[0] += PBI * PBW
                w2f_tiles[c] = w2f

            w2_casted = set()

            def emit_w2_cast(c):
                if c >= NCH or c in w2_casted:
                    return
                w2_casted.add(c)
                if c in w2s_tiles:
                    w2s8 = w2s_tiles[c]
                    w2f = w2f_tiles[c]
                    for i_l in range(PBI):
                        cast(w2f[:, i_l, :], w2s8[:, i_l, :])

            # ---------------- Phase A: h = silu(x@w1^T * s1) * (x@w3^T * s3)
            # software-pipelined emission: DMA group g+2, cast group g+1,
            # then compute group g, so casts never wait behind chain ops.
            f3off = [0]
            q3off = [0]
            w1f_t = {}
            w3f_t = {}
            w3s_t = {}
            w3_casted = set()

            def emit_w13_dma(g):
                if g >= len(GWS) or g in w1f_t:
                    return
                gw = GWS[g]
                goff = sum(GWS[:g])
                w1f = w1p.tile([P, KD, GWMAX], f16, tag="w1f")
                nc.sync.dma_start(
                    w1f[:, :, :gw],
                    w1t[:, goff * KD:(goff + gw) * KD]
                    .rearrange("p (k f) -> p k f", f=gw))
                w1f_t[g] = w1f
                w3f = w3fp.tile([P, KD, GWMAX], f16, tag="w3f")
                if W3_I8[g]:
                    w3s8 = w3sp.tile([P, KD, GWMAX], i8, tag="w3s8")
                    nc.scalar.dma_start(
                        w3s8[:, :, :gw],
                        w3tq[:, q3off[0]:q3off[0] + gw * KD]
                        .rearrange("p (k f) -> p k f", f=gw))
                    q3off[0] += gw * KD
                    w3s_t[g] = w3s8
                else:
                    nc.scalar.dma_start(
                        w3f[:, :, :gw],
                        w3tf[:, f3off[0]:f3off[0] + gw * KD]
                        .rearrange("p (k f) -> p k f", f=gw))
                    f3off[0] += gw * KD
                w3f_t[g] = w3f

            def emit_w3_cast(g):
                if g >= len(GWS) or g in w3_casted:
                    return
                w3_casted.add(g)
                if g in w3s_t:
                    gw = GWS[g]
                    for k in range(KD):
                        cast(w3f_t[g][:, k, :gw], w3s_t[g][:, k, :gw])

            with tc.tile_pool(name="psA", bufs=4, space="PSUM") as psA:
                emit_w13_dma(0)
                emit_w13_dma(1)
                emit_w3_cast(0)
                goff = 0
                for g, gw in enumerate(GWS):
                    il0 = goff // P
                    nil = gw // P
                    emit_w13_dma(g + 2)
                    emit_w3_cast(g + 1)
                    # overlap part of phase B's weight traffic with phase A
                    if g >= 10 and len(w2f_tiles) < PREFETCH_CHUNKS:
                        emit_w2_dma(len(w2f_tiles))
                    w1f = w1f_t[g]
                    w3f = w3f_t[g]

                    il = 0
                    while il < nil:
                        jn = min(jn_max, nil - il)
                        i = il0 + il
                        p1 = psA.tile([P, JSZ, C], f32, tag="p1")
                        p3 = psA.tile([P, JSZ, C], f32, tag="p3")
                        for j in range(jn):
                            lo = (il + j) * P
                            for k in range(KD):
                                nc.tensor.matmul(
                                    p1[:, j, :], w1f[:, k, lo:lo + P],
                                    xts[:, k, :],
                                    start=(k == 0), stop=(k == KD - 1))
                        for j in range(jn):
                            lo = (il + j) * P
                            for k in range(KD):
                                nc.tensor.matmul(
                                    p3[:, j, :], w3f[:, k, lo:lo + P],
                                    xts[:, k, :],
                                    start=(k == 0), stop=(k == KD - 1))
                        t1 = ep.tile([P, JSZ, C], f16, tag="t1")
                        psmul(t1[:, :jn, :], p1[:, :jn, :], s1s, i, jn)
                        a = ep.tile([P, JSZ, C], f16, tag="a")
                        nc.scalar.activation(
                            a[:, :jn, :], t1[:, :jn, :],
                            mybir.ActivationFunctionType.Silu)
                        acc["act"] += 128 * jn * C / 55 + 400
                        t3 = ep.tile([P, JSZ, C], f16, tag="t3")
                        psmul(t3[:, :jn, :], p3[:, :jn, :], s3s, i, jn)
                        htile = hp.tile([P, JSZ, C], f16, tag=f"h{i}")
                        mul(htile[:, :jn, :], t3[:, :jn, :], a[:, :jn, :])
                        for j in range(jn):
                            h_index[i + j] = (htile, j)
                        il += jn
                    goff += gw

            # ---------------- Phase B: y^T = (w2 @ h) * s2
            with tc.tile_pool(name="psB", bufs=2, space="PSUM") as psB:
                for mh in range(MH):
                    pbs = [psB.tile([P, C], f32, tag=f"pb{ml}",
                                    name=f"pb{mh}_{ml}")
                           for ml in range(PBM)]
                    for nb in range(NB):
                        c = mh * NB + nb
                        emit_w2_dma(c)       # no-op unless pipeline fell behind
                        emit_w2_cast(c)
                        emit_w2_dma(c + 2)
                        emit_w2_cast(c + 1)
                        w2f = w2f_tiles[c]
                        for i_l in range(PBI):
                            i = nb * PBI + i_l
                            ht, j = h_index[i]
                            for ml in range(PBM):
                                nc.tensor.matmul(
                                    pbs[ml],
                                    w2f[:, i_l, ml * P:(ml + 1) * P],
                                    ht[:, j, :],
                                    start=(i == 0), stop=(i == KI - 1))
                    for ml in range(PBM):
                        m = mh * PBM + ml
                        o = outp.tile([P, C], f32, tag="o")
                        if ml % 2 == 0:
                            nc.vector.tensor_scalar_mul(
                                o, pbs[ml], s2s[:, m:m + 1])
                        else:
                            nc.scalar.mul(o, pbs[ml], s2s[:, m:m + 1])
                        q = nc.sync if ml % 2 == 0 else nc.scalar
                        q.dma_start(yt[m * P:(m + 1) * P, :], o)

    nc.compile()
    return nc


def _tile_w13(w, dtype_sel):
    """[I, D] -> (fp16 blocks, int8 blocks) packed per GWS/dtype_sel."""
    fparts, qparts = [], []
    goff = 0
    for gw, is_i8 in zip(GWS, dtype_sel):
        blk = w[goff:goff + gw, :]                      # [gw, D] int8
        t = blk.reshape(gw, KD, P).transpose(2, 1, 0).reshape(P, KD * gw)
        if is_i8:
            qparts.append(t)
        else:
            fparts.append(t.astype(np.float16))
        goff += gw
    fcat = (np.ascontiguousarray(np.concatenate(fparts, axis=1))
            if fparts else np.zeros((P, 0), np.float16))
    qcat = (np.ascontiguousarray(np.concatenate(qparts, axis=1))
            if qparts else np.zeros((P, 0), np.int8))
    return fcat, qcat


def _tile_w2(w):
    """[D, I] int8 -> (fp16 chunks, int8 chunks) packed per W2_I8."""
    t = w.reshape(MH, PBW, KI, P).transpose(0, 3, 2, 1)  # [MH, P, KI, PBW]
    fparts, qparts = [], []
    for mh in range(MH):
        for nb in range(NB):
            c = t[mh, :, nb * PBI:(nb + 1) * PBI, :].reshape(P, PBI * PBW)
            if W2_I8[mh][nb]:
                qparts.append(c)
            else:
                fparts.append(c.astype(np.float16))
    fcat = (np.ascontiguousarray(np.concatenate(fparts, axis=1))
            if fparts else np.zeros((P, 0), np.float16))
    qcat = (np.ascontiguousarray(np.concatenate(qparts, axis=1))
            if qparts else np.zeros((P, 0), np.int8))
    return fcat, qcat


def _route(expert_indices):
    idx = np.asarray(expert_indices).astype(np.int64)
    toks, slots = [], []
    for e in range(E):
        t, a = np.nonzero(idx == e)
        toks.append(t)
        slots.append(a)
    return toks, slots


def _prepare(inputs):
    x = np.asarray(inputs["x"], dtype=np.float32)          # [T, D]
    expert_indices = np.asarray(inputs["expert_indices"])  # [T, A]
    w1 = np.asarray(inputs["w1"])                          # [E, I, D] int8
    w2 = np.asarray(inputs["w2"])                          # [E, D, I] int8
    w3 = np.asarray(inputs["w3"])                          # [E, I, D] int8
    scales1 = np.asarray(inputs["scales1"], dtype=np.float32)
    scales2 = np.asarray(inputs["scales2"], dtype=np.float32)
    scales3 = np.asarray(inputs["scales3"], dtype=np.float32)

    T, A = expert_indices.shape
    toks, slots = _route(expert_indices)
    counts = [len(t) for t in toks]
    C = max(max(counts), 8)
    C = (C + 1) // 2 * 2

    if C not in _CACHE:
        _CACHE[C] = _build_nc(C)
    nc = _CACHE[C]

    in_maps = []
    for e in range(E):
        n_e = counts[e]
        xtc = np.zeros((P, KD, C), dtype=np.float16)
        if n_e:
            xe = x[toks[e]].astype(np.float16)             # [n_e, D]
            xtc[:, :, :n_e] = xe.T.reshape(KD, P, n_e).transpose(1, 0, 2)
        w1f, _ = _tile_w13(w1[e], [False] * len(GWS))
        w3f, w3q = _tile_w13(w3[e], W3_I8)
        w2f, w2q = _tile_w2(w2[e])
        in_maps.append(
            dict(
                xt=np.ascontiguousarray(xtc.reshape(P, KD * C)),
                w1t=w1f,
                w3tf=w3f,
                w3tq=w3q,
                w2tf=w2f,
                w2tq=w2q,
                s1=np.ascontiguousarray(scales1[e].reshape(KI, P).T),
                s3=np.ascontiguousarray(scales3[e].reshape(KI, P).T),
                s2=np.ascontiguousarray(scales2[e].reshape(MT, P).T),
            )
        )
    return nc, in_maps, (T, A, toks, slots, counts)


def kernel(**inputs):
    global _LAST_RESULTS
    from concourse.bass_utils import run_bass_kernel_spmd

    nc, in_maps, (T, A, toks, slots, counts) = _prepare(inputs)
    res = run_bass_kernel_spmd(nc, in_maps, core_ids=list(range(E)))
    _LAST_RESULTS = res

    out = np.zeros((T, A, D), dtype=np.float32)
    for e in range(E):
        n_e = counts[e]
        if n_e:
            ye = res.results[e]["yt"][:, :n_e].T  # [n_e, D]
            out[toks[e], slots[e], :] = ye
    return out
